# revision 34
# baseline (speedup 1.0000x reference)
"""LSTM cell (B=4096, D=U=2048) on 8 trn2 NeuronCores.

Tensor-parallel over units: core i computes units [i*256,(i+1)*256) of every
gate. Per core:
    z^T[units, 4096 batch] = Wx_shard^T @ x^T + Wh_shard^T @ h^T
Gates f,i (and the first two k-pairs of gate o) run as fp8e4 DoubleRow
matmuls (2 k-tiles per instruction, 2x PE rate; f,i weights pre-scaled by
S=1024 on the host, 1/S folded into the gate activation's scale operand —
gate o's psum mixes fp8 and bf16 contributions, so its fp8 path splits the
scale: weights*8 against dedicated activations/8). Gates o,g otherwise stay
bf16 — the tanh gate g dominates the error budget, so this split lands at
~1.96e-2 rel err against the 2e-2 gate while cutting PE work to ~0.71x. Accumulation is fp32
in PSUM; gate activations fuse the bias add (units on partitions -> bias is
per-partition) on ScalarE; elementwise LSTM combine on VectorE; outputs
stored transposed and re-transposed on the host.

All activation/weight tensors are host pre-tiled so each DMA reads long
contiguous per-partition lines, and DMA issue (~0.6us each on the issuing
sequencer) is split across both HWDGE queues: activations on SP (sync),
weights + c-state + output stores on Activation (scalar), ordered so the
first matmul only waits on the first x8 chunk + a 128KB weight chunk.
"""

import sys

sys.path.insert(0, "/opt/trn_rl_repo")

import ml_dtypes
import numpy as np

import concourse.bass as bass
import concourse.mybir as mybir
import concourse.tile as tile
from concourse.bass_utils import run_bass_kernel_spmd

B, D, U = 4096, 2048, 2048
N_CORES = 8
US = U // N_CORES          # units per core per gate (256)
UT = US // 128             # unit tiles of 128 per gate (2)
NB = 512                   # batch tile (free dim)
NT = B // NB               # batch tiles (8)
KT = D // 128              # k tiles per operand gemm (16)
KP = KT // 2               # fp8 DoubleRow k-tile pairs (8)
SW = 1024.0                # fp8 weight scale (absmax*SW ~ 122 < 240)
SO = 8.0                   # gate-o fp8 split scale (w*SO, acts/SO)
# chunk lists: one dma_start per chunk. Each dma_start lands on a single
# HW queue served round-robin (~22GB/s/queue when all are busy), so startup
# arrival latency wants small chunks across many queues, while sequencer
# issue cost (~0.6us per dma_start) wants few — these sizes balance the two.
CHUNKS = [(0, 2), (2, 4), (4, 8), (8, 16)]             # act chunks (k-tiles)
WFI_CH = [(0, 1), (1, 4), (4, 8)]                      # f,i weights (pairs)
WOG_CH = [(0, 4), (4, 8), (8, 12), (12, 16)]           # o,g weights (k-tiles)
BF16 = mybir.dt.bfloat16
F8 = mybir.dt.float8e4
F32 = mybir.dt.float32
AF = mybir.ActivationFunctionType
DR = mybir.MatmulPerfMode.DoubleRow

# gate index: 0=f, 1=i (fp8, block [f|i]); 2=o, 3=g (bf16, block [o|g])
# bias column for (gate, ut) = 2*gate + ut


def _split_excess_waits(nc, maxw=1):
    """This walrus build rejects instructions carrying more than one sem-wait
    ("Too many sync wait commands"), but Tile freely attaches several. Hoist
    the extra waits onto same-engine nops inserted right before the
    instruction — engine streams are in-order, so blocking semantics are
    identical."""
    cnt = 0
    for fn in nc.m.functions:
        for bb in fn.blocks:
            new_insts = []
            for inst in bb.instructions:
                si = inst.sync_info
                waits = list(si.on_wait) if si is not None else []
                if len(waits) > maxw:
                    for i in range(0, len(waits) - maxw, maxw):
                        nop = mybir.InstNoOp(name=f"syncsplit-{cnt}")
                        cnt += 1
                        nop.engine = inst.engine
                        nop.sync_info = mybir.SyncInfo(
                            on_wait=waits[i : i + maxw], on_update=[]
                        )
                        new_insts.append(nop)
                    si.on_wait = waits[len(waits) - maxw :]
                new_insts.append(inst)
            if len(new_insts) != len(bb.instructions):
                bb.instructions = new_insts
    return cnt


def build_nc() -> bass.Bass:
    nc = bass.Bass()

    # activations pre-tiled to [batch-tile, partition, k-tile, batch-col]
    xT = nc.dram_tensor("xT", [NT, 128, KT, NB], BF16, kind="ExternalInput")
    hT = nc.dram_tensor("hT", [NT, 128, KT, NB], BF16, kind="ExternalInput")
    x8T = nc.dram_tensor("x8T", [NT, 128, KT, NB], F8, kind="ExternalInput")
    h8T = nc.dram_tensor("h8T", [NT, 128, KT, NB], F8, kind="ExternalInput")
    # weights pre-tiled to [partition, k-tile, unit-col]
    wxog = nc.dram_tensor("wxog", [128, KT, 2 * US], BF16, kind="ExternalInput")
    whog = nc.dram_tensor("whog", [128, KT, 2 * US], BF16, kind="ExternalInput")
    wxfi = nc.dram_tensor("wxfi", [128, KT, 2 * US], F8, kind="ExternalInput")
    whfi = nc.dram_tensor("whfi", [128, KT, 2 * US], F8, kind="ExternalInput")
    # gate-o fp8 path for k-tiles 0..3 (pairs 0,1): weights scaled by SO and
    # dedicated activations pre-scaled by 1/SO, so the product lands
    # unscaled in the psum it shares with bf16 contributions while both
    # operands stay clear of fp8 denormals
    wxo8 = nc.dram_tensor("wxo8", [128, 4, US], F8, kind="ExternalInput")
    who8 = nc.dram_tensor("who8", [128, 4, US], F8, kind="ExternalInput")
    x8oT = nc.dram_tensor("x8oT", [NT, 128, 4, NB], F8, kind="ExternalInput")
    h8oT = nc.dram_tensor("h8oT", [NT, 128, 4, NB], F8, kind="ExternalInput")
    # bias, host-prepped to [128, 8]: column 2*gate+ut, gate order [f,i,o,g]
    bias = nc.dram_tensor("bias", [128, 4 * UT], F32, kind="ExternalInput")
    cT = nc.dram_tensor("cT", [US, B], F32, kind="ExternalInput")
    h_newT = nc.dram_tensor("h_newT", [US, B], F32, kind="ExternalOutput")
    c_newT = nc.dram_tensor("c_newT", [US, B], F32, kind="ExternalOutput")

    with tile.TileContext(nc) as tc:
        with (
            tc.tile_pool(name="wpool", bufs=1) as wpool,
            tc.tile_pool(name="singles", bufs=1) as singles,
            tc.tile_pool(name="acts", bufs=2) as apool,
            tc.tile_pool(name="ew", bufs=2) as epool,
            tc.tile_pool(name="psum", bufs=8, space="PSUM") as ppool,
        ):
            b_sb = singles.tile([128, 4 * UT], F32)

            def load_chunks(src, n, dt, tagp):
                ts_ = []
                for ci, (k0, k1) in enumerate(CHUNKS):
                    t = apool.tile([128, k1 - k0, NB], dt, tag=f"{tagp}{ci}")
                    nc.sync.dma_start(out=t[:], in_=src[n, :, k0:k1, :])
                    ts_.append(t)
                return ts_

            def load_w(src, dt, tagp, bounds, pair, eng=None):
                eng = eng or nc.scalar
                ts_ = []
                for ci, (c0, c1) in enumerate(bounds):
                    m = 2 if pair else 1
                    t = wpool.tile([128, (c1 - c0) * m, 2 * US], dt, tag=f"{tagp}{ci}")
                    eng.dma_start(out=t[:], in_=src[:, c0 * m : c1 * m, :])
                    ts_.append(t)
                return ts_

            def chunk_of(bounds, idx):
                for ci, (k0, k1) in enumerate(bounds):
                    if k0 <= idx < k1:
                        return ci, idx - k0
                raise AssertionError

            # --- startup. Sync queue: x8/x/h8/h chunk loads in PE phase
            # order. Scalar queue: weights, a tiny first fp8 chunk first so
            # the opening matmul unblocks as early as possible.
            def load_o8(src, n, tagp):
                t = apool.tile([128, 4, NB], F8, tag=tagp)
                nc.sync.dma_start(out=t[:], in_=src[n])
                return t

            # PE clock warmup: the Tensor engine ramps through low/mid
            # pstates over its first ~3us of execution. The first real
            # matmul's data lands ~11.5us in and warmup can start ~9.2us in
            # (engine boot + memset), so a handful of garbage matmuls fill
            # exactly that window and hand off at speed; the first real
            # group resets the bank with start=True.
            warm = singles.tile([128, NB], BF16, name="warm")
            nc.scalar.memzero(warm[:])
            ps_warm = ppool.tile([128, NB], F32, tag="ps", name="ps_warm")
            for _ in range(5):
                nc.tensor.matmul(
                    ps_warm[:],
                    warm[:, 0:128],
                    warm[:],
                    start=True,
                    stop=True,
                    skip_group_check=True,
                )

            x8q = load_chunks(x8T, 0, F8, "x8q")
            wxfi_t = load_w(wxfi, F8, "wxfi", WFI_CH, pair=True)
            wo8_sb = singles.tile([128, 4, US], F8, name="wxo8")
            nc.scalar.dma_start(out=wo8_sb[:], in_=wxo8[:])
            xq = load_chunks(xT, 0, BF16, "xq")
            wxog_t = load_w(wxog, BF16, "wxog", WOG_CH, pair=False)
            # phase D's bulk (hq + whog, 3MB) posts before phase C's small
            # fp8 load so its queue share starts early enough to arrive by
            # the time phase D opens (~43us); phase C's 1.5MB still clears.
            hq = load_chunks(hT, 0, BF16, "hq")
            whog_t = load_w(whog, BF16, "whog", WOG_CH, pair=False)
            h8q = load_chunks(h8T, 0, F8, "h8q")
            x8o_t = load_o8(x8oT, 0, "x8o")
            h8o_t = load_o8(h8oT, 0, "h8o")
            whfi_t = load_w(whfi, F8, "whfi", WFI_CH, pair=True)
            who8_sb = singles.tile([128, 4, US], F8, name="who8")
            nc.scalar.dma_start(out=who8_sb[:], in_=who8[:])
            nc.scalar.dma_start(out=b_sb[:], in_=bias[:])

            def mm_fp8(ps, w_t, aq, gate, ut, j, start, stop):
                c0 = (gate == 1) * US + ut * 128
                wc, wr = chunk_of(WFI_CH, j)
                ac, ar = chunk_of(CHUNKS, 2 * j)
                nc.tensor.matmul(
                    ps[:],
                    w_t[wc][:, 2 * wr : 2 * wr + 2, c0 : c0 + 128],
                    aq[ac][:, ar : ar + 2, :],
                    start=start,
                    stop=stop,
                    perf_mode=DR,
                )

            def mm_o8(ps, w8_sb, a8o, ut, m, start, stop):
                # gate-o DoubleRow over k-pair m (k-tiles 2m, 2m+1)
                nc.tensor.matmul(
                    ps[:],
                    w8_sb[:, 2 * m : 2 * m + 2, ut * 128 : (ut + 1) * 128],
                    a8o[:, 2 * m : 2 * m + 2, :],
                    start=start,
                    stop=stop,
                    perf_mode=DR,
                )

            def mm_bf16(ps, w_t, aq, gate, ut, kt, start, stop):
                c0 = (gate == 3) * US + ut * 128
                wc, wr = chunk_of(WOG_CH, kt)
                ac, ar = chunk_of(CHUNKS, kt)
                nc.tensor.matmul(
                    ps[:],
                    w_t[wc][:, wr, c0 : c0 + 128],
                    aq[ac][:, ar, :],
                    start=start,
                    stop=stop,
                )

            def act_gate(pss, gate, ut, name):
                g_sb = epool.tile([128, NB], F32, tag=f"gate{gate}", name=name)
                nc.scalar.activation(
                    g_sb[:],
                    pss[gate][:],
                    AF.Tanh if gate == 3 else AF.Sigmoid,
                    bias=b_sb[:, 2 * gate + ut : 2 * gate + ut + 1],
                    scale=(1.0 / SW) if gate <= 1 else 1.0,
                )
                return g_sb

            def elementwise(pss, n, ut, fi_first):
                nsl = bass.ts(n, NB)
                usl = slice(ut * 128, (ut + 1) * 128)
                c_sb = epool.tile([128, NB], F32, tag="c_sb", name="c_sb")
                nc.scalar.dma_start(out=c_sb[:], in_=cT[usl, nsl])
                if fi_first:
                    i_t = act_gate(pss, 1, ut, "i_t")
                    f_t = act_gate(pss, 0, ut, "f_t")
                    nc.vector.tensor_mul(f_t[:], f_t[:], c_sb[:])   # f*c
                    g_t = act_gate(pss, 3, ut, "g_t")
                    nc.vector.tensor_mul(i_t[:], i_t[:], g_t[:])    # i*g
                else:
                    g_t = act_gate(pss, 3, ut, "g_t")
                    i_t = act_gate(pss, 1, ut, "i_t")
                    nc.vector.tensor_mul(i_t[:], i_t[:], g_t[:])    # i*g
                    f_t = act_gate(pss, 0, ut, "f_t")
                    nc.vector.tensor_mul(f_t[:], f_t[:], c_sb[:])   # f*c
                cn = epool.tile([128, NB], F32, tag="cn", name="cn")
                nc.vector.tensor_add(cn[:], f_t[:], i_t[:])         # c_new
                nc.scalar.dma_start(out=c_newT[usl, nsl], in_=cn[:])
                nc.scalar.activation(g_t[:], cn[:], AF.Tanh)        # tanh(c_new)
                o_t = act_gate(pss, 2, ut, "o_t")
                nc.vector.tensor_mul(o_t[:], o_t[:], g_t[:])        # h_new
                nc.scalar.dma_start(out=h_newT[usl, nsl], in_=o_t[:])

            # --- n = 0: k-outer inside each of four phases (fp8-x, bf16-x,
            # fp8-h, bf16-h) matching the DMA arrival stream; all 8 PSUM
            # groups held open across phases. Gate o opens with its fp8
            # k-pair-0 DoubleRow in the fp8 phases.
            ps_all = [
                [
                    ppool.tile([128, NB], F32, tag="ps", name=f"ps{ut}{g}")
                    for g in range(4)
                ]
                for ut in range(UT)
            ]
            for j in range(KP):
                for ut in range(UT):
                    for g in (1, 0):
                        mm_fp8(ps_all[ut][g], wxfi_t, x8q, g, ut, j, j == 0, False)
            for kt in range(KT):
                for ut in range(UT):
                    mm_bf16(ps_all[ut][3], wxog_t, xq, 3, ut, kt, kt == 0, False)
                    if kt >= 4:
                        mm_bf16(ps_all[ut][2], wxog_t, xq, 2, ut, kt, kt == 4, False)
            for j in range(KP):
                for ut in range(UT):
                    for g in (1, 0):
                        mm_fp8(ps_all[ut][g], whfi_t, h8q, g, ut, j, False, j == KP - 1)
            for ut in range(UT):
                for m in range(2):
                    mm_o8(ps_all[ut][2], wo8_sb, x8o_t, ut, m, False, False)
                    mm_o8(ps_all[ut][2], who8_sb, h8o_t, ut, m, False, False)
            for kt in range(KT):
                for ut in range(UT):
                    mm_bf16(ps_all[ut][3], whog_t, hq, 3, ut, kt, False, kt == KT - 1)
                    if kt >= 4:
                        mm_bf16(ps_all[ut][2], whog_t, hq, 2, ut, kt, False, kt == KT - 1)
            for ut in range(UT):
                elementwise(ps_all[ut], 0, ut, fi_first=True)

            # --- n = 1..7: gate-outer, k-inner; order g (bf16), i (fp8),
            # f (fp8), o (fp8 pair 0 + bf16 rest) so the elementwise chain
            # consumes gates in completion order and only o's short tail
            # trails the matmuls.
            for n in range(1, NT):
                xq = load_chunks(xT, n, BF16, "xq")
                hq = load_chunks(hT, n, BF16, "hq")
                x8q = load_chunks(x8T, n, F8, "x8q")
                h8q = load_chunks(h8T, n, F8, "h8q")
                x8o_t = load_o8(x8oT, n, "x8o")
                h8o_t = load_o8(h8oT, n, "h8o")
                for ut in range(UT):
                    pss = [
                        ppool.tile([128, NB], F32, tag="ps", name=f"ps{g}")
                        for g in range(4)
                    ]
                    for kt in range(KT):
                        mm_bf16(pss[3], wxog_t, xq, 3, ut, kt, kt == 0, False)
                    for kt in range(KT):
                        mm_bf16(pss[3], whog_t, hq, 3, ut, kt, False, kt == KT - 1)
                    for j in range(KP):
                        mm_fp8(pss[1], wxfi_t, x8q, 1, ut, j, j == 0, False)
                    for j in range(KP):
                        mm_fp8(pss[1], whfi_t, h8q, 1, ut, j, False, j == KP - 1)
                    for j in range(KP):
                        mm_fp8(pss[0], wxfi_t, x8q, 0, ut, j, j == 0, False)
                    for j in range(KP):
                        mm_fp8(pss[0], whfi_t, h8q, 0, ut, j, False, j == KP - 1)
                    for m in range(2):
                        mm_o8(pss[2], wo8_sb, x8o_t, ut, m, m == 0, False)
                        mm_o8(pss[2], who8_sb, h8o_t, ut, m, False, False)
                    for kt in range(4, KT):
                        mm_bf16(pss[2], wxog_t, xq, 2, ut, kt, False, False)
                    for kt in range(4, KT):
                        mm_bf16(pss[2], whog_t, hq, 2, ut, kt, False, kt == KT - 1)
                    elementwise(pss, n, ut, fi_first=False)
    _split_excess_waits(nc)
    return nc


_NC_CACHE = None


def _get_nc():
    global _NC_CACHE
    if _NC_CACHE is None:
        _NC_CACHE = build_nc()
    return _NC_CACHE


def _tile_act(aT):
    """[D, B] -> [NT, 128, KT, NB] with contiguous per-partition lines."""
    return np.ascontiguousarray(
        aT.reshape(KT, 128, NT, NB).transpose(2, 1, 0, 3)
    )


def _tile_w(w):
    """[D, F] -> [128, KT', F] with contiguous per-partition lines."""
    return np.ascontiguousarray(
        w.reshape(w.shape[0] // 128, 128, w.shape[1]).transpose(1, 0, 2)
    )


def make_in_maps(x, h, c, Wxf, Wxi, Wxo, Wxg, bf, bi, bo, bg, Whf, Whi, Who, Whg):
    bf16 = ml_dtypes.bfloat16
    f8 = ml_dtypes.float8_e4m3
    xTt = _tile_act(np.ascontiguousarray(np.asarray(x, np.float32).T))
    hTt = _tile_act(np.ascontiguousarray(np.asarray(h, np.float32).T))
    xT = xTt.astype(bf16)
    hT = hTt.astype(bf16)
    x8T = xTt.astype(f8)
    h8T = hTt.astype(f8)
    x8oT = (xTt[:, :, 0:4, :] / SO).astype(f8)
    h8oT = (hTt[:, :, 0:4, :] / SO).astype(f8)
    c = np.asarray(c, np.float32)
    Wx = {k: np.asarray(w, np.float32) for k, w in
          zip("fiog", (Wxf, Wxi, Wxo, Wxg))}
    Wh = {k: np.asarray(w, np.float32) for k, w in
          zip("fiog", (Whf, Whi, Who, Whg))}
    bv = {k: np.asarray(v, np.float32) for k, v in zip("fiog", (bf, bi, bo, bg))}

    in_maps = []
    for i in range(N_CORES):
        s = slice(i * US, (i + 1) * US)
        wxog_i = _tile_w(np.concatenate([Wx["o"][:, s], Wx["g"][:, s]], 1)).astype(bf16)
        whog_i = _tile_w(np.concatenate([Wh["o"][:, s], Wh["g"][:, s]], 1)).astype(bf16)
        wxfi_i = _tile_w(
            np.concatenate([Wx["f"][:, s], Wx["i"][:, s]], 1) * SW
        ).astype(f8)
        whfi_i = _tile_w(
            np.concatenate([Wh["f"][:, s], Wh["i"][:, s]], 1) * SW
        ).astype(f8)
        # gate-o fp8 weights, k rows 0..511 (pairs 0,1): psum mixes with
        # unscaled bf16 contributions, so split the scale with the dedicated
        # o activations: w*SO here, acts/SO above — product unscaled, both
        # operands clear of fp8 denormals
        wxo8_i = _tile_w(Wx["o"][:512, s] * SO).astype(f8)
        who8_i = _tile_w(Wh["o"][:512, s] * SO).astype(f8)
        # bias [128, 8]: col 2*gate+ut, gate order [f,i,o,g]
        b_i = np.concatenate([bv[k][s] for k in "fiog"])
        b_i = np.ascontiguousarray(b_i.reshape(4 * UT, 128).T)
        cT_i = np.ascontiguousarray(c[:, s].T)
        in_maps.append(
            {
                "xT": xT, "hT": hT, "x8T": x8T, "h8T": h8T,
                "x8oT": x8oT, "h8oT": h8oT,
                "wxog": wxog_i, "whog": whog_i,
                "wxfi": wxfi_i, "whfi": whfi_i,
                "wxo8": wxo8_i, "who8": who8_i,
                "bias": b_i, "cT": cT_i,
            }
        )
    return in_maps


def run(in_maps, **kwargs):
    nc = _get_nc()
    return run_bass_kernel_spmd(nc, in_maps, list(range(N_CORES)), **kwargs)


def gather(results):
    h_new = np.empty((B, U), np.float32)
    c_new = np.empty((B, U), np.float32)
    for i in range(N_CORES):
        s = slice(i * US, (i + 1) * US)
        h_new[:, s] = results[i]["h_newT"].T
        c_new[:, s] = results[i]["c_newT"].T
    return h_new, c_new


def kernel(**inputs):
    res = run(make_in_maps(**inputs))
    return gather(res.results)


# revision 35
# speedup vs baseline: 1.0023x; 1.0023x over previous
"""LSTM cell (B=4096, D=U=2048) on 8 trn2 NeuronCores.

Tensor-parallel over units: core i computes units [i*256,(i+1)*256) of every
gate. Per core:
    z^T[units, 4096 batch] = Wx_shard^T @ x^T + Wh_shard^T @ h^T
Gates f,i (and the first two k-pairs of gate o) run as fp8e4 DoubleRow
matmuls (2 k-tiles per instruction, 2x PE rate; f,i weights pre-scaled by
S=1024 on the host, 1/S folded into the gate activation's scale operand —
gate o's psum mixes fp8 and bf16 contributions, so its fp8 path splits the
scale: weights*8 against dedicated activations/8). Gates o,g otherwise stay
bf16 — the tanh gate g dominates the error budget, so this split lands at
~1.96e-2 rel err against the 2e-2 gate while cutting PE work to ~0.71x. Accumulation is fp32
in PSUM; gate activations fuse the bias add (units on partitions -> bias is
per-partition) on ScalarE; elementwise LSTM combine on VectorE; outputs
stored transposed and re-transposed on the host.

All activation/weight tensors are host pre-tiled so each DMA reads long
contiguous per-partition lines, and DMA issue (~0.6us each on the issuing
sequencer) is split across both HWDGE queues: activations on SP (sync),
weights + c-state + output stores on Activation (scalar), ordered so the
first matmul only waits on the first x8 chunk + a 128KB weight chunk.
"""

import sys

sys.path.insert(0, "/opt/trn_rl_repo")

import ml_dtypes
import numpy as np

import concourse.bass as bass
import concourse.mybir as mybir
import concourse.tile as tile
from concourse.bass_utils import run_bass_kernel_spmd

B, D, U = 4096, 2048, 2048
N_CORES = 8
US = U // N_CORES          # units per core per gate (256)
UT = US // 128             # unit tiles of 128 per gate (2)
NB = 512                   # batch tile (free dim)
NT = B // NB               # batch tiles (8)
KT = D // 128              # k tiles per operand gemm (16)
KP = KT // 2               # fp8 DoubleRow k-tile pairs (8)
SW = 1024.0                # fp8 weight scale (absmax*SW ~ 122 < 240)
SO = 8.0                   # gate-o fp8 split scale (w*SO, acts/SO)
# chunk lists: one dma_start per chunk. Each dma_start lands on a single
# HW queue served round-robin (~22GB/s/queue when all are busy), so startup
# arrival latency wants small chunks across many queues, while sequencer
# issue cost (~0.6us per dma_start) wants few — these sizes balance the two.
CHUNKS = [(0, 2), (2, 4), (4, 8), (8, 16)]             # act chunks (k-tiles)
WFI_CH = [(0, 1), (1, 4), (4, 8)]                      # f,i weights (pairs)
WOG_CH = [(0, 4), (4, 8), (8, 12), (12, 16)]           # o,g weights (k-tiles)
BF16 = mybir.dt.bfloat16
F8 = mybir.dt.float8e4
F32 = mybir.dt.float32
AF = mybir.ActivationFunctionType
DR = mybir.MatmulPerfMode.DoubleRow

# gate index: 0=f, 1=i (fp8, block [f|i]); 2=o, 3=g (bf16, block [o|g])
# bias column for (gate, ut) = 2*gate + ut


def _split_excess_waits(nc, maxw=1):
    """This walrus build rejects instructions carrying more than one sem-wait
    ("Too many sync wait commands"), but Tile freely attaches several. Hoist
    the extra waits onto same-engine nops inserted right before the
    instruction — engine streams are in-order, so blocking semantics are
    identical."""
    cnt = 0
    for fn in nc.m.functions:
        for bb in fn.blocks:
            new_insts = []
            for inst in bb.instructions:
                si = inst.sync_info
                waits = list(si.on_wait) if si is not None else []
                if len(waits) > maxw:
                    for i in range(0, len(waits) - maxw, maxw):
                        nop = mybir.InstNoOp(name=f"syncsplit-{cnt}")
                        cnt += 1
                        nop.engine = inst.engine
                        nop.sync_info = mybir.SyncInfo(
                            on_wait=waits[i : i + maxw], on_update=[]
                        )
                        new_insts.append(nop)
                    si.on_wait = waits[len(waits) - maxw :]
                new_insts.append(inst)
            if len(new_insts) != len(bb.instructions):
                bb.instructions = new_insts
    return cnt


def build_nc() -> bass.Bass:
    nc = bass.Bass()

    # activations pre-tiled to [batch-tile, partition, k-tile, batch-col]
    xT = nc.dram_tensor("xT", [NT, 128, KT, NB], BF16, kind="ExternalInput")
    hT = nc.dram_tensor("hT", [NT, 128, KT, NB], BF16, kind="ExternalInput")
    x8T = nc.dram_tensor("x8T", [NT, 128, KT, NB], F8, kind="ExternalInput")
    h8T = nc.dram_tensor("h8T", [NT, 128, KT, NB], F8, kind="ExternalInput")
    # weights pre-tiled to [partition, k-tile, unit-col]
    wxog = nc.dram_tensor("wxog", [128, KT, 2 * US], BF16, kind="ExternalInput")
    whog = nc.dram_tensor("whog", [128, KT, 2 * US], BF16, kind="ExternalInput")
    wxfi = nc.dram_tensor("wxfi", [128, KT, 2 * US], F8, kind="ExternalInput")
    whfi = nc.dram_tensor("whfi", [128, KT, 2 * US], F8, kind="ExternalInput")
    # gate-o fp8 path for k-tiles 0..3 (pairs 0,1): weights scaled by SO and
    # dedicated activations pre-scaled by 1/SO, so the product lands
    # unscaled in the psum it shares with bf16 contributions while both
    # operands stay clear of fp8 denormals
    wxo8 = nc.dram_tensor("wxo8", [128, 4, US], F8, kind="ExternalInput")
    who8 = nc.dram_tensor("who8", [128, 4, US], F8, kind="ExternalInput")
    x8oT = nc.dram_tensor("x8oT", [NT, 128, 4, NB], F8, kind="ExternalInput")
    h8oT = nc.dram_tensor("h8oT", [NT, 128, 4, NB], F8, kind="ExternalInput")
    # bias, host-prepped to [128, 8]: column 2*gate+ut, gate order [f,i,o,g]
    bias = nc.dram_tensor("bias", [128, 4 * UT], F32, kind="ExternalInput")
    cT = nc.dram_tensor("cT", [US, B], F32, kind="ExternalInput")
    h_newT = nc.dram_tensor("h_newT", [US, B], F32, kind="ExternalOutput")
    c_newT = nc.dram_tensor("c_newT", [US, B], F32, kind="ExternalOutput")

    with tile.TileContext(nc) as tc:
        with (
            tc.tile_pool(name="wpool", bufs=1) as wpool,
            tc.tile_pool(name="singles", bufs=1) as singles,
            tc.tile_pool(name="acts", bufs=2) as apool,
            tc.tile_pool(name="ew", bufs=2) as epool,
            tc.tile_pool(name="psum", bufs=8, space="PSUM") as ppool,
        ):
            b_sb = singles.tile([128, 4 * UT], F32)

            def load_chunks(src, n, dt, tagp):
                ts_ = []
                for ci, (k0, k1) in enumerate(CHUNKS):
                    t = apool.tile([128, k1 - k0, NB], dt, tag=f"{tagp}{ci}")
                    nc.sync.dma_start(out=t[:], in_=src[n, :, k0:k1, :])
                    ts_.append(t)
                return ts_

            def load_w(src, dt, tagp, bounds, pair, eng=None):
                eng = eng or nc.scalar
                ts_ = []
                for ci, (c0, c1) in enumerate(bounds):
                    m = 2 if pair else 1
                    t = wpool.tile([128, (c1 - c0) * m, 2 * US], dt, tag=f"{tagp}{ci}")
                    eng.dma_start(out=t[:], in_=src[:, c0 * m : c1 * m, :])
                    ts_.append(t)
                return ts_

            def chunk_of(bounds, idx):
                for ci, (k0, k1) in enumerate(bounds):
                    if k0 <= idx < k1:
                        return ci, idx - k0
                raise AssertionError

            # --- startup. Sync queue: x8/x/h8/h chunk loads in PE phase
            # order. Scalar queue: weights, a tiny first fp8 chunk first so
            # the opening matmul unblocks as early as possible.
            def load_o8(src, n, tagp):
                t = apool.tile([128, 4, NB], F8, tag=tagp)
                nc.sync.dma_start(out=t[:], in_=src[n])
                return t

            x8q = load_chunks(x8T, 0, F8, "x8q")
            wxfi_t = load_w(wxfi, F8, "wxfi", WFI_CH, pair=True)
            wo8_sb = singles.tile([128, 4, US], F8, name="wxo8")
            nc.scalar.dma_start(out=wo8_sb[:], in_=wxo8[:])
            xq = load_chunks(xT, 0, BF16, "xq")
            wxog_t = load_w(wxog, BF16, "wxog", WOG_CH, pair=False)
            # phase D's bulk (hq + whog, 3MB) posts before phase C's small
            # fp8 load so its queue share starts early enough to arrive by
            # the time phase D opens (~43us); phase C's 1.5MB still clears.
            hq = load_chunks(hT, 0, BF16, "hq")
            whog_t = load_w(whog, BF16, "whog", WOG_CH, pair=False)
            h8q = load_chunks(h8T, 0, F8, "h8q")
            x8o_t = load_o8(x8oT, 0, "x8o")
            h8o_t = load_o8(h8oT, 0, "h8o")
            whfi_t = load_w(whfi, F8, "whfi", WFI_CH, pair=True)
            who8_sb = singles.tile([128, 4, US], F8, name="who8")
            nc.scalar.dma_start(out=who8_sb[:], in_=who8[:])
            nc.scalar.dma_start(out=b_sb[:], in_=bias[:])

            def mm_fp8(ps, w_t, aq, gate, ut, j, start, stop):
                c0 = (gate == 1) * US + ut * 128
                wc, wr = chunk_of(WFI_CH, j)
                ac, ar = chunk_of(CHUNKS, 2 * j)
                nc.tensor.matmul(
                    ps[:],
                    w_t[wc][:, 2 * wr : 2 * wr + 2, c0 : c0 + 128],
                    aq[ac][:, ar : ar + 2, :],
                    start=start,
                    stop=stop,
                    perf_mode=DR,
                )

            def mm_o8(ps, w8_sb, a8o, ut, m, start, stop):
                # gate-o DoubleRow over k-pair m (k-tiles 2m, 2m+1)
                nc.tensor.matmul(
                    ps[:],
                    w8_sb[:, 2 * m : 2 * m + 2, ut * 128 : (ut + 1) * 128],
                    a8o[:, 2 * m : 2 * m + 2, :],
                    start=start,
                    stop=stop,
                    perf_mode=DR,
                )

            def mm_bf16(ps, w_t, aq, gate, ut, kt, start, stop):
                c0 = (gate == 3) * US + ut * 128
                wc, wr = chunk_of(WOG_CH, kt)
                ac, ar = chunk_of(CHUNKS, kt)
                nc.tensor.matmul(
                    ps[:],
                    w_t[wc][:, wr, c0 : c0 + 128],
                    aq[ac][:, ar, :],
                    start=start,
                    stop=stop,
                )

            def act_gate(pss, gate, ut, name):
                g_sb = epool.tile([128, NB], F32, tag=f"gate{gate}", name=name)
                nc.scalar.activation(
                    g_sb[:],
                    pss[gate][:],
                    AF.Tanh if gate == 3 else AF.Sigmoid,
                    bias=b_sb[:, 2 * gate + ut : 2 * gate + ut + 1],
                    scale=(1.0 / SW) if gate <= 1 else 1.0,
                )
                return g_sb

            def elementwise(pss, n, ut, fi_first):
                nsl = bass.ts(n, NB)
                usl = slice(ut * 128, (ut + 1) * 128)
                c_sb = epool.tile([128, NB], F32, tag="c_sb", name="c_sb")
                nc.scalar.dma_start(out=c_sb[:], in_=cT[usl, nsl])
                if fi_first:
                    i_t = act_gate(pss, 1, ut, "i_t")
                    f_t = act_gate(pss, 0, ut, "f_t")
                    nc.vector.tensor_mul(f_t[:], f_t[:], c_sb[:])   # f*c
                    g_t = act_gate(pss, 3, ut, "g_t")
                    nc.vector.tensor_mul(i_t[:], i_t[:], g_t[:])    # i*g
                else:
                    g_t = act_gate(pss, 3, ut, "g_t")
                    i_t = act_gate(pss, 1, ut, "i_t")
                    nc.vector.tensor_mul(i_t[:], i_t[:], g_t[:])    # i*g
                    f_t = act_gate(pss, 0, ut, "f_t")
                    nc.vector.tensor_mul(f_t[:], f_t[:], c_sb[:])   # f*c
                cn = epool.tile([128, NB], F32, tag="cn", name="cn")
                nc.vector.tensor_add(cn[:], f_t[:], i_t[:])         # c_new
                nc.scalar.dma_start(out=c_newT[usl, nsl], in_=cn[:])
                nc.scalar.activation(g_t[:], cn[:], AF.Tanh)        # tanh(c_new)
                o_t = act_gate(pss, 2, ut, "o_t")
                nc.vector.tensor_mul(o_t[:], o_t[:], g_t[:])        # h_new
                nc.scalar.dma_start(out=h_newT[usl, nsl], in_=o_t[:])

            # --- n = 0: k-outer inside each of four phases (fp8-x, bf16-x,
            # fp8-h, bf16-h) matching the DMA arrival stream; all 8 PSUM
            # groups held open across phases. Gate o opens with its fp8
            # k-pair-0 DoubleRow in the fp8 phases.
            ps_all = [
                [
                    ppool.tile([128, NB], F32, tag="ps", name=f"ps{ut}{g}")
                    for g in range(4)
                ]
                for ut in range(UT)
            ]
            for j in range(KP):
                for ut in range(UT):
                    for g in (1, 0):
                        mm_fp8(ps_all[ut][g], wxfi_t, x8q, g, ut, j, j == 0, False)
            for kt in range(KT):
                for ut in range(UT):
                    mm_bf16(ps_all[ut][3], wxog_t, xq, 3, ut, kt, kt == 0, False)
                    if kt >= 4:
                        mm_bf16(ps_all[ut][2], wxog_t, xq, 2, ut, kt, kt == 4, False)
            for j in range(KP):
                for ut in range(UT):
                    for g in (1, 0):
                        mm_fp8(ps_all[ut][g], whfi_t, h8q, g, ut, j, False, j == KP - 1)
            for ut in range(UT):
                for m in range(2):
                    mm_o8(ps_all[ut][2], wo8_sb, x8o_t, ut, m, False, False)
                    mm_o8(ps_all[ut][2], who8_sb, h8o_t, ut, m, False, False)
            for kt in range(KT):
                for ut in range(UT):
                    mm_bf16(ps_all[ut][3], whog_t, hq, 3, ut, kt, False, kt == KT - 1)
                    if kt >= 4:
                        mm_bf16(ps_all[ut][2], whog_t, hq, 2, ut, kt, False, kt == KT - 1)
            for ut in range(UT):
                elementwise(ps_all[ut], 0, ut, fi_first=True)

            # --- n = 1..7: gate-outer, k-inner; order g (bf16), i (fp8),
            # f (fp8), o (fp8 pair 0 + bf16 rest) so the elementwise chain
            # consumes gates in completion order and only o's short tail
            # trails the matmuls.
            for n in range(1, NT):
                xq = load_chunks(xT, n, BF16, "xq")
                hq = load_chunks(hT, n, BF16, "hq")
                x8q = load_chunks(x8T, n, F8, "x8q")
                h8q = load_chunks(h8T, n, F8, "h8q")
                x8o_t = load_o8(x8oT, n, "x8o")
                h8o_t = load_o8(h8oT, n, "h8o")
                for ut in range(UT):
                    pss = [
                        ppool.tile([128, NB], F32, tag="ps", name=f"ps{g}")
                        for g in range(4)
                    ]
                    for kt in range(KT):
                        mm_bf16(pss[3], wxog_t, xq, 3, ut, kt, kt == 0, False)
                    for kt in range(KT):
                        mm_bf16(pss[3], whog_t, hq, 3, ut, kt, False, kt == KT - 1)
                    for j in range(KP):
                        mm_fp8(pss[1], wxfi_t, x8q, 1, ut, j, j == 0, False)
                    for j in range(KP):
                        mm_fp8(pss[1], whfi_t, h8q, 1, ut, j, False, j == KP - 1)
                    for j in range(KP):
                        mm_fp8(pss[0], wxfi_t, x8q, 0, ut, j, j == 0, False)
                    for j in range(KP):
                        mm_fp8(pss[0], whfi_t, h8q, 0, ut, j, False, j == KP - 1)
                    for m in range(2):
                        mm_o8(pss[2], wo8_sb, x8o_t, ut, m, m == 0, False)
                        mm_o8(pss[2], who8_sb, h8o_t, ut, m, False, False)
                    for kt in range(4, KT):
                        mm_bf16(pss[2], wxog_t, xq, 2, ut, kt, False, False)
                    for kt in range(4, KT):
                        mm_bf16(pss[2], whog_t, hq, 2, ut, kt, False, kt == KT - 1)
                    elementwise(pss, n, ut, fi_first=False)
    _split_excess_waits(nc)
    return nc


_NC_CACHE = None


def _get_nc():
    global _NC_CACHE
    if _NC_CACHE is None:
        _NC_CACHE = build_nc()
    return _NC_CACHE


def _tile_act(aT):
    """[D, B] -> [NT, 128, KT, NB] with contiguous per-partition lines."""
    return np.ascontiguousarray(
        aT.reshape(KT, 128, NT, NB).transpose(2, 1, 0, 3)
    )


def _tile_w(w):
    """[D, F] -> [128, KT', F] with contiguous per-partition lines."""
    return np.ascontiguousarray(
        w.reshape(w.shape[0] // 128, 128, w.shape[1]).transpose(1, 0, 2)
    )


def make_in_maps(x, h, c, Wxf, Wxi, Wxo, Wxg, bf, bi, bo, bg, Whf, Whi, Who, Whg):
    bf16 = ml_dtypes.bfloat16
    f8 = ml_dtypes.float8_e4m3
    xTt = _tile_act(np.ascontiguousarray(np.asarray(x, np.float32).T))
    hTt = _tile_act(np.ascontiguousarray(np.asarray(h, np.float32).T))
    xT = xTt.astype(bf16)
    hT = hTt.astype(bf16)
    x8T = xTt.astype(f8)
    h8T = hTt.astype(f8)
    x8oT = (xTt[:, :, 0:4, :] / SO).astype(f8)
    h8oT = (hTt[:, :, 0:4, :] / SO).astype(f8)
    c = np.asarray(c, np.float32)
    Wx = {k: np.asarray(w, np.float32) for k, w in
          zip("fiog", (Wxf, Wxi, Wxo, Wxg))}
    Wh = {k: np.asarray(w, np.float32) for k, w in
          zip("fiog", (Whf, Whi, Who, Whg))}
    bv = {k: np.asarray(v, np.float32) for k, v in zip("fiog", (bf, bi, bo, bg))}

    in_maps = []
    for i in range(N_CORES):
        s = slice(i * US, (i + 1) * US)
        wxog_i = _tile_w(np.concatenate([Wx["o"][:, s], Wx["g"][:, s]], 1)).astype(bf16)
        whog_i = _tile_w(np.concatenate([Wh["o"][:, s], Wh["g"][:, s]], 1)).astype(bf16)
        wxfi_i = _tile_w(
            np.concatenate([Wx["f"][:, s], Wx["i"][:, s]], 1) * SW
        ).astype(f8)
        whfi_i = _tile_w(
            np.concatenate([Wh["f"][:, s], Wh["i"][:, s]], 1) * SW
        ).astype(f8)
        # gate-o fp8 weights, k rows 0..511 (pairs 0,1): psum mixes with
        # unscaled bf16 contributions, so split the scale with the dedicated
        # o activations: w*SO here, acts/SO above — product unscaled, both
        # operands clear of fp8 denormals
        wxo8_i = _tile_w(Wx["o"][:512, s] * SO).astype(f8)
        who8_i = _tile_w(Wh["o"][:512, s] * SO).astype(f8)
        # bias [128, 8]: col 2*gate+ut, gate order [f,i,o,g]
        b_i = np.concatenate([bv[k][s] for k in "fiog"])
        b_i = np.ascontiguousarray(b_i.reshape(4 * UT, 128).T)
        cT_i = np.ascontiguousarray(c[:, s].T)
        in_maps.append(
            {
                "xT": xT, "hT": hT, "x8T": x8T, "h8T": h8T,
                "x8oT": x8oT, "h8oT": h8oT,
                "wxog": wxog_i, "whog": whog_i,
                "wxfi": wxfi_i, "whfi": whfi_i,
                "wxo8": wxo8_i, "who8": who8_i,
                "bias": b_i, "cT": cT_i,
            }
        )
    return in_maps


def run(in_maps, **kwargs):
    nc = _get_nc()
    return run_bass_kernel_spmd(nc, in_maps, list(range(N_CORES)), **kwargs)


def gather(results):
    h_new = np.empty((B, U), np.float32)
    c_new = np.empty((B, U), np.float32)
    for i in range(N_CORES):
        s = slice(i * US, (i + 1) * US)
        h_new[:, s] = results[i]["h_newT"].T
        c_new[:, s] = results[i]["c_newT"].T
    return h_new, c_new


def kernel(**inputs):
    res = run(make_in_maps(**inputs))
    return gather(res.results)


# revision 37
# speedup vs baseline: 1.0039x; 1.0016x over previous
"""LSTM cell (B=4096, D=U=2048) on 8 trn2 NeuronCores.

Tensor-parallel over units: core i computes units [i*256,(i+1)*256) of every
gate. Per core:
    z^T[units, 4096 batch] = Wx_shard^T @ x^T + Wh_shard^T @ h^T
Gates f,i (and the first two k-pairs of gate o) run as fp8e4 DoubleRow
matmuls (2 k-tiles per instruction, 2x PE rate; f,i weights pre-scaled by
S=1024 on the host, 1/S folded into the gate activation's scale operand —
gate o's psum mixes fp8 and bf16 contributions, so its fp8 path splits the
scale: weights*8 against dedicated activations/8). Gates o,g otherwise stay
bf16 — the tanh gate g dominates the error budget, so this split lands at
~1.96e-2 rel err against the 2e-2 gate while cutting PE work to ~0.71x. Accumulation is fp32
in PSUM; gate activations fuse the bias add (units on partitions -> bias is
per-partition) on ScalarE; elementwise LSTM combine on VectorE; outputs
stored transposed and re-transposed on the host.

All activation/weight tensors are host pre-tiled so each DMA reads long
contiguous per-partition lines, and DMA issue (~0.6us each on the issuing
sequencer) is split across both HWDGE queues: activations on SP (sync),
weights + c-state + output stores on Activation (scalar), ordered so the
first matmul only waits on the first x8 chunk + a 128KB weight chunk.
"""

import sys

sys.path.insert(0, "/opt/trn_rl_repo")

import ml_dtypes
import numpy as np

import concourse.bass as bass
import concourse.mybir as mybir
import concourse.tile as tile
from concourse.bass_utils import run_bass_kernel_spmd

B, D, U = 4096, 2048, 2048
N_CORES = 8
US = U // N_CORES          # units per core per gate (256)
UT = US // 128             # unit tiles of 128 per gate (2)
NB = 512                   # batch tile (free dim)
NT = B // NB               # batch tiles (8)
KT = D // 128              # k tiles per operand gemm (16)
KP = KT // 2               # fp8 DoubleRow k-tile pairs (8)
SW = 1024.0                # fp8 weight scale (absmax*SW ~ 122 < 240)
SO = 8.0                   # gate-o fp8 split scale (w*SO, acts/SO)
# chunk lists: one dma_start per chunk. Each dma_start lands on a single
# HW queue served round-robin (~22GB/s/queue when all are busy), so startup
# arrival latency wants small chunks across many queues, while sequencer
# issue cost (~0.6us per dma_start) wants few — these sizes balance the two.
CHUNKS = [(0, 2), (2, 4), (4, 8), (8, 16)]             # act chunks (k-tiles)
WFI_CH = [(0, 1), (1, 4), (4, 8)]                      # f,i weights (pairs)
WOG_CH = [(0, 4), (4, 8), (8, 12), (12, 16)]           # o,g weights (k-tiles)
BF16 = mybir.dt.bfloat16
F8 = mybir.dt.float8e4
F32 = mybir.dt.float32
AF = mybir.ActivationFunctionType
DR = mybir.MatmulPerfMode.DoubleRow

# gate index: 0=f, 1=i (fp8, block [f|i]); 2=o, 3=g (bf16, block [o|g])
# bias column for (gate, ut) = 2*gate + ut


def _split_excess_waits(nc, maxw=1):
    """This walrus build rejects instructions carrying more than one sem-wait
    ("Too many sync wait commands"), but Tile freely attaches several. Hoist
    the extra waits onto same-engine nops inserted right before the
    instruction — engine streams are in-order, so blocking semantics are
    identical."""
    cnt = 0
    for fn in nc.m.functions:
        for bb in fn.blocks:
            new_insts = []
            for inst in bb.instructions:
                si = inst.sync_info
                waits = list(si.on_wait) if si is not None else []
                if len(waits) > maxw:
                    for i in range(0, len(waits) - maxw, maxw):
                        nop = mybir.InstNoOp(name=f"syncsplit-{cnt}")
                        cnt += 1
                        nop.engine = inst.engine
                        nop.sync_info = mybir.SyncInfo(
                            on_wait=waits[i : i + maxw], on_update=[]
                        )
                        new_insts.append(nop)
                    si.on_wait = waits[len(waits) - maxw :]
                new_insts.append(inst)
            if len(new_insts) != len(bb.instructions):
                bb.instructions = new_insts
    return cnt


def build_nc() -> bass.Bass:
    nc = bass.Bass()

    # activations pre-tiled to [batch-tile, partition, k-tile, batch-col]
    xT = nc.dram_tensor("xT", [NT, 128, KT, NB], BF16, kind="ExternalInput")
    hT = nc.dram_tensor("hT", [NT, 128, KT, NB], BF16, kind="ExternalInput")
    x8T = nc.dram_tensor("x8T", [NT, 128, KT, NB], F8, kind="ExternalInput")
    h8T = nc.dram_tensor("h8T", [NT, 128, KT, NB], F8, kind="ExternalInput")
    # weights pre-tiled to [partition, k-tile, unit-col]
    wxog = nc.dram_tensor("wxog", [128, KT, 2 * US], BF16, kind="ExternalInput")
    whog = nc.dram_tensor("whog", [128, KT, 2 * US], BF16, kind="ExternalInput")
    wxfi = nc.dram_tensor("wxfi", [128, KT, 2 * US], F8, kind="ExternalInput")
    whfi = nc.dram_tensor("whfi", [128, KT, 2 * US], F8, kind="ExternalInput")
    # gate-o fp8 path for k-tiles 0..3 (pairs 0,1): weights scaled by SO and
    # dedicated activations pre-scaled by 1/SO, so the product lands
    # unscaled in the psum it shares with bf16 contributions while both
    # operands stay clear of fp8 denormals
    wxo8 = nc.dram_tensor("wxo8", [128, 4, US], F8, kind="ExternalInput")
    who8 = nc.dram_tensor("who8", [128, 4, US], F8, kind="ExternalInput")
    x8oT = nc.dram_tensor("x8oT", [NT, 128, 4, NB], F8, kind="ExternalInput")
    h8oT = nc.dram_tensor("h8oT", [NT, 128, 4, NB], F8, kind="ExternalInput")
    # bias, host-prepped to [128, 8]: column 2*gate+ut, gate order [f,i,o,g]
    bias = nc.dram_tensor("bias", [128, 4 * UT], F32, kind="ExternalInput")
    cT = nc.dram_tensor("cT", [US, B], F32, kind="ExternalInput")
    h_newT = nc.dram_tensor("h_newT", [US, B], F32, kind="ExternalOutput")
    c_newT = nc.dram_tensor("c_newT", [US, B], F32, kind="ExternalOutput")

    with tile.TileContext(nc) as tc:
        with (
            tc.tile_pool(name="wpool", bufs=1) as wpool,
            tc.tile_pool(name="singles", bufs=1) as singles,
            tc.tile_pool(name="acts", bufs=2) as apool,
            tc.tile_pool(name="ew", bufs=2) as epool,
            tc.tile_pool(name="psum", bufs=8, space="PSUM") as ppool,
        ):
            b_sb = singles.tile([128, 4 * UT], F32)

            def load_chunks(src, n, dt, tagp):
                ts_ = []
                for ci, (k0, k1) in enumerate(CHUNKS):
                    t = apool.tile([128, k1 - k0, NB], dt, tag=f"{tagp}{ci}")
                    nc.sync.dma_start(out=t[:], in_=src[n, :, k0:k1, :])
                    ts_.append(t)
                return ts_

            def load_w(src, dt, tagp, bounds, pair, eng=None):
                eng = eng or nc.scalar
                ts_ = []
                for ci, (c0, c1) in enumerate(bounds):
                    m = 2 if pair else 1
                    t = wpool.tile([128, (c1 - c0) * m, 2 * US], dt, tag=f"{tagp}{ci}")
                    eng.dma_start(out=t[:], in_=src[:, c0 * m : c1 * m, :])
                    ts_.append(t)
                return ts_

            def chunk_of(bounds, idx):
                for ci, (k0, k1) in enumerate(bounds):
                    if k0 <= idx < k1:
                        return ci, idx - k0
                raise AssertionError

            # --- startup. Sync queue: x8/x/h8/h chunk loads in PE phase
            # order. Scalar queue: weights, a tiny first fp8 chunk first so
            # the opening matmul unblocks as early as possible.
            def load_o8(src, n, tagp):
                t = apool.tile([128, 4, NB], F8, tag=tagp)
                nc.sync.dma_start(out=t[:], in_=src[n])
                return t

            # the opening matmul waits on x8 chunk 0 + wxfi chunk 0: split
            # each across two queues to halve their arrival latency
            x8q = []
            t = apool.tile([128, 2, NB], F8, tag="x8q0")
            nc.sync.dma_start(out=t[:, 0:1, :], in_=x8T[0, :, 0:1, :])
            nc.sync.dma_start(out=t[:, 1:2, :], in_=x8T[0, :, 1:2, :])
            x8q.append(t)
            for ci, (k0, k1) in enumerate(CHUNKS[1:], start=1):
                t = apool.tile([128, k1 - k0, NB], F8, tag=f"x8q{ci}")
                nc.sync.dma_start(out=t[:], in_=x8T[0, :, k0:k1, :])
                x8q.append(t)
            wxfi_t = []
            t = wpool.tile([128, 2, 2 * US], F8, tag="wxfi0")
            nc.scalar.dma_start(out=t[:, 0:1, :], in_=wxfi[:, 0:1, :])
            nc.scalar.dma_start(out=t[:, 1:2, :], in_=wxfi[:, 1:2, :])
            wxfi_t.append(t)
            for ci, (p0, p1) in enumerate(WFI_CH[1:], start=1):
                t = wpool.tile([128, (p1 - p0) * 2, 2 * US], F8, tag=f"wxfi{ci}")
                nc.scalar.dma_start(out=t[:], in_=wxfi[:, p0 * 2 : p1 * 2, :])
                wxfi_t.append(t)
            wo8_sb = singles.tile([128, 4, US], F8, name="wxo8")
            nc.scalar.dma_start(out=wo8_sb[:], in_=wxo8[:])
            xq = load_chunks(xT, 0, BF16, "xq")
            wxog_t = load_w(wxog, BF16, "wxog", WOG_CH, pair=False)
            # phase D's bulk (hq + whog, 3MB) posts before phase C's small
            # fp8 load so its queue share starts early enough to arrive by
            # the time phase D opens (~43us); phase C's 1.5MB still clears.
            hq = load_chunks(hT, 0, BF16, "hq")
            whog_t = load_w(whog, BF16, "whog", WOG_CH, pair=False)
            h8q = load_chunks(h8T, 0, F8, "h8q")
            x8o_t = load_o8(x8oT, 0, "x8o")
            h8o_t = load_o8(h8oT, 0, "h8o")
            whfi_t = load_w(whfi, F8, "whfi", WFI_CH, pair=True)
            who8_sb = singles.tile([128, 4, US], F8, name="who8")
            nc.scalar.dma_start(out=who8_sb[:], in_=who8[:])
            nc.scalar.dma_start(out=b_sb[:], in_=bias[:])

            def mm_fp8(ps, w_t, aq, gate, ut, j, start, stop):
                c0 = (gate == 1) * US + ut * 128
                wc, wr = chunk_of(WFI_CH, j)
                ac, ar = chunk_of(CHUNKS, 2 * j)
                nc.tensor.matmul(
                    ps[:],
                    w_t[wc][:, 2 * wr : 2 * wr + 2, c0 : c0 + 128],
                    aq[ac][:, ar : ar + 2, :],
                    start=start,
                    stop=stop,
                    perf_mode=DR,
                )

            def mm_o8(ps, w8_sb, a8o, ut, m, start, stop):
                # gate-o DoubleRow over k-pair m (k-tiles 2m, 2m+1)
                nc.tensor.matmul(
                    ps[:],
                    w8_sb[:, 2 * m : 2 * m + 2, ut * 128 : (ut + 1) * 128],
                    a8o[:, 2 * m : 2 * m + 2, :],
                    start=start,
                    stop=stop,
                    perf_mode=DR,
                )

            def mm_bf16(ps, w_t, aq, gate, ut, kt, start, stop):
                c0 = (gate == 3) * US + ut * 128
                wc, wr = chunk_of(WOG_CH, kt)
                ac, ar = chunk_of(CHUNKS, kt)
                nc.tensor.matmul(
                    ps[:],
                    w_t[wc][:, wr, c0 : c0 + 128],
                    aq[ac][:, ar, :],
                    start=start,
                    stop=stop,
                )

            def act_gate(pss, gate, ut, name):
                g_sb = epool.tile([128, NB], F32, tag=f"gate{gate}", name=name)
                nc.scalar.activation(
                    g_sb[:],
                    pss[gate][:],
                    AF.Tanh if gate == 3 else AF.Sigmoid,
                    bias=b_sb[:, 2 * gate + ut : 2 * gate + ut + 1],
                    scale=(1.0 / SW) if gate <= 1 else 1.0,
                )
                return g_sb

            def elementwise(pss, n, ut, fi_first):
                nsl = bass.ts(n, NB)
                usl = slice(ut * 128, (ut + 1) * 128)
                c_sb = epool.tile([128, NB], F32, tag="c_sb", name="c_sb")
                nc.scalar.dma_start(out=c_sb[:], in_=cT[usl, nsl])
                if fi_first:
                    i_t = act_gate(pss, 1, ut, "i_t")
                    f_t = act_gate(pss, 0, ut, "f_t")
                    nc.vector.tensor_mul(f_t[:], f_t[:], c_sb[:])   # f*c
                    g_t = act_gate(pss, 3, ut, "g_t")
                    nc.vector.tensor_mul(i_t[:], i_t[:], g_t[:])    # i*g
                else:
                    g_t = act_gate(pss, 3, ut, "g_t")
                    i_t = act_gate(pss, 1, ut, "i_t")
                    nc.vector.tensor_mul(i_t[:], i_t[:], g_t[:])    # i*g
                    f_t = act_gate(pss, 0, ut, "f_t")
                    nc.vector.tensor_mul(f_t[:], f_t[:], c_sb[:])   # f*c
                cn = epool.tile([128, NB], F32, tag="cn", name="cn")
                nc.vector.tensor_add(cn[:], f_t[:], i_t[:])         # c_new
                nc.scalar.dma_start(out=c_newT[usl, nsl], in_=cn[:])
                nc.scalar.activation(g_t[:], cn[:], AF.Tanh)        # tanh(c_new)
                o_t = act_gate(pss, 2, ut, "o_t")
                nc.vector.tensor_mul(o_t[:], o_t[:], g_t[:])        # h_new
                if n == NT - 1 and ut == UT - 1:
                    # the very last store is the serial tail of the kernel:
                    # split it across two queues (one per HWDGE engine)
                    nb0 = n * NB
                    nc.scalar.dma_start(
                        out=h_newT[usl, nb0 : nb0 + NB // 2],
                        in_=o_t[:, 0 : NB // 2],
                    )
                    nc.sync.dma_start(
                        out=h_newT[usl, nb0 + NB // 2 : nb0 + NB],
                        in_=o_t[:, NB // 2 : NB],
                    )
                else:
                    nc.scalar.dma_start(out=h_newT[usl, nsl], in_=o_t[:])

            # --- n = 0: k-outer inside each of four phases (fp8-x, bf16-x,
            # fp8-h, bf16-h) matching the DMA arrival stream; all 8 PSUM
            # groups held open across phases. Gate o opens with its fp8
            # k-pair-0 DoubleRow in the fp8 phases.
            ps_all = [
                [
                    ppool.tile([128, NB], F32, tag="ps", name=f"ps{ut}{g}")
                    for g in range(4)
                ]
                for ut in range(UT)
            ]
            for j in range(KP):
                for ut in range(UT):
                    for g in (1, 0):
                        mm_fp8(ps_all[ut][g], wxfi_t, x8q, g, ut, j, j == 0, False)
            for kt in range(KT):
                for ut in range(UT):
                    mm_bf16(ps_all[ut][3], wxog_t, xq, 3, ut, kt, kt == 0, False)
                    if kt >= 4:
                        mm_bf16(ps_all[ut][2], wxog_t, xq, 2, ut, kt, kt == 4, False)
            for j in range(KP):
                for ut in range(UT):
                    for g in (1, 0):
                        mm_fp8(ps_all[ut][g], whfi_t, h8q, g, ut, j, False, j == KP - 1)
            for ut in range(UT):
                for m in range(2):
                    mm_o8(ps_all[ut][2], wo8_sb, x8o_t, ut, m, False, False)
                    mm_o8(ps_all[ut][2], who8_sb, h8o_t, ut, m, False, False)
            for kt in range(KT):
                for ut in range(UT):
                    mm_bf16(ps_all[ut][3], whog_t, hq, 3, ut, kt, False, kt == KT - 1)
                    if kt >= 4:
                        mm_bf16(ps_all[ut][2], whog_t, hq, 2, ut, kt, False, kt == KT - 1)
            for ut in range(UT):
                elementwise(ps_all[ut], 0, ut, fi_first=True)

            # --- n = 1..7: gate-outer, k-inner; order g (bf16), i (fp8),
            # f (fp8), o (fp8 pair 0 + bf16 rest) so the elementwise chain
            # consumes gates in completion order and only o's short tail
            # trails the matmuls.
            for n in range(1, NT):
                xq = load_chunks(xT, n, BF16, "xq")
                hq = load_chunks(hT, n, BF16, "hq")
                x8q = load_chunks(x8T, n, F8, "x8q")
                h8q = load_chunks(h8T, n, F8, "h8q")
                x8o_t = load_o8(x8oT, n, "x8o")
                h8o_t = load_o8(h8oT, n, "h8o")
                for ut in range(UT):
                    pss = [
                        ppool.tile([128, NB], F32, tag="ps", name=f"ps{g}")
                        for g in range(4)
                    ]
                    for kt in range(KT):
                        mm_bf16(pss[3], wxog_t, xq, 3, ut, kt, kt == 0, False)
                    for kt in range(KT):
                        mm_bf16(pss[3], whog_t, hq, 3, ut, kt, False, kt == KT - 1)
                    for j in range(KP):
                        mm_fp8(pss[1], wxfi_t, x8q, 1, ut, j, j == 0, False)
                    for j in range(KP):
                        mm_fp8(pss[1], whfi_t, h8q, 1, ut, j, False, j == KP - 1)
                    for j in range(KP):
                        mm_fp8(pss[0], wxfi_t, x8q, 0, ut, j, j == 0, False)
                    for j in range(KP):
                        mm_fp8(pss[0], whfi_t, h8q, 0, ut, j, False, j == KP - 1)
                    for m in range(2):
                        mm_o8(pss[2], wo8_sb, x8o_t, ut, m, m == 0, False)
                        mm_o8(pss[2], who8_sb, h8o_t, ut, m, False, False)
                    for kt in range(4, KT):
                        mm_bf16(pss[2], wxog_t, xq, 2, ut, kt, False, False)
                    for kt in range(4, KT):
                        mm_bf16(pss[2], whog_t, hq, 2, ut, kt, False, kt == KT - 1)
                    elementwise(pss, n, ut, fi_first=False)
    _split_excess_waits(nc)
    return nc


_NC_CACHE = None


def _get_nc():
    global _NC_CACHE
    if _NC_CACHE is None:
        _NC_CACHE = build_nc()
    return _NC_CACHE


def _tile_act(aT):
    """[D, B] -> [NT, 128, KT, NB] with contiguous per-partition lines."""
    return np.ascontiguousarray(
        aT.reshape(KT, 128, NT, NB).transpose(2, 1, 0, 3)
    )


def _tile_w(w):
    """[D, F] -> [128, KT', F] with contiguous per-partition lines."""
    return np.ascontiguousarray(
        w.reshape(w.shape[0] // 128, 128, w.shape[1]).transpose(1, 0, 2)
    )


def make_in_maps(x, h, c, Wxf, Wxi, Wxo, Wxg, bf, bi, bo, bg, Whf, Whi, Who, Whg):
    bf16 = ml_dtypes.bfloat16
    f8 = ml_dtypes.float8_e4m3
    xTt = _tile_act(np.ascontiguousarray(np.asarray(x, np.float32).T))
    hTt = _tile_act(np.ascontiguousarray(np.asarray(h, np.float32).T))
    xT = xTt.astype(bf16)
    hT = hTt.astype(bf16)
    x8T = xTt.astype(f8)
    h8T = hTt.astype(f8)
    x8oT = (xTt[:, :, 0:4, :] / SO).astype(f8)
    h8oT = (hTt[:, :, 0:4, :] / SO).astype(f8)
    c = np.asarray(c, np.float32)
    Wx = {k: np.asarray(w, np.float32) for k, w in
          zip("fiog", (Wxf, Wxi, Wxo, Wxg))}
    Wh = {k: np.asarray(w, np.float32) for k, w in
          zip("fiog", (Whf, Whi, Who, Whg))}
    bv = {k: np.asarray(v, np.float32) for k, v in zip("fiog", (bf, bi, bo, bg))}

    in_maps = []
    for i in range(N_CORES):
        s = slice(i * US, (i + 1) * US)
        wxog_i = _tile_w(np.concatenate([Wx["o"][:, s], Wx["g"][:, s]], 1)).astype(bf16)
        whog_i = _tile_w(np.concatenate([Wh["o"][:, s], Wh["g"][:, s]], 1)).astype(bf16)
        wxfi_i = _tile_w(
            np.concatenate([Wx["f"][:, s], Wx["i"][:, s]], 1) * SW
        ).astype(f8)
        whfi_i = _tile_w(
            np.concatenate([Wh["f"][:, s], Wh["i"][:, s]], 1) * SW
        ).astype(f8)
        # gate-o fp8 weights, k rows 0..511 (pairs 0,1): psum mixes with
        # unscaled bf16 contributions, so split the scale with the dedicated
        # o activations: w*SO here, acts/SO above — product unscaled, both
        # operands clear of fp8 denormals
        wxo8_i = _tile_w(Wx["o"][:512, s] * SO).astype(f8)
        who8_i = _tile_w(Wh["o"][:512, s] * SO).astype(f8)
        # bias [128, 8]: col 2*gate+ut, gate order [f,i,o,g]
        b_i = np.concatenate([bv[k][s] for k in "fiog"])
        b_i = np.ascontiguousarray(b_i.reshape(4 * UT, 128).T)
        cT_i = np.ascontiguousarray(c[:, s].T)
        in_maps.append(
            {
                "xT": xT, "hT": hT, "x8T": x8T, "h8T": h8T,
                "x8oT": x8oT, "h8oT": h8oT,
                "wxog": wxog_i, "whog": whog_i,
                "wxfi": wxfi_i, "whfi": whfi_i,
                "wxo8": wxo8_i, "who8": who8_i,
                "bias": b_i, "cT": cT_i,
            }
        )
    return in_maps


def run(in_maps, **kwargs):
    nc = _get_nc()
    return run_bass_kernel_spmd(nc, in_maps, list(range(N_CORES)), **kwargs)


def gather(results):
    h_new = np.empty((B, U), np.float32)
    c_new = np.empty((B, U), np.float32)
    for i in range(N_CORES):
        s = slice(i * US, (i + 1) * US)
        h_new[:, s] = results[i]["h_newT"].T
        c_new[:, s] = results[i]["c_newT"].T
    return h_new, c_new


def kernel(**inputs):
    res = run(make_in_maps(**inputs))
    return gather(res.results)
